# revision 16
# baseline (speedup 1.0000x reference)
"""Trainium2 Bass kernel for nn_BARO_89318139887969 (topk_masking).

Computation per (b, w):
  readout = mean_n(x) + max_n(x)                      [h]
  e = gelu(BN(readout @ W_embed + b_embed))           [h]   (BN folded into W', b')
  gate = sigmoid(e @ W_attend + b_attend)             [n]
  output = mean_n(x * gate)                           [h]
  selected = sum over top-10-gate rows of (x * gate)  [h]
  out3 = gate transposed to [w, b, n]

Sharding: data-parallel over batch b across 8 cores (1 batch each).
Per-core layout: x_b [32, 400, 256] processed in 16 pairs of windows,
each pair's x in SBUF as [100, (t=2, c=4, h=256)] (n split into 4 chunks
of 100 partitions).

Compute engines can neither shift nor stride partitions, and fp32/f32r
matmuls reject tile_position; so per-window PSUM rows are produced via
zero-padded one-hot weight columns (zeros accumulate harmlessly into the
other rows), keeping everything at base partition 0.

Engine split (per window):
  DMA : x pair load (contiguous 1KB rows)
  Act : fp32 -> f32r rounding cast (major share), PSUM evacuations, erf/sigmoid
  DVE : cross-chunk max fold (exact fp32), topk (hw top-8 + match_replace)
  Pool: partition_all_reduce max over n, cast share, output DMA issue
  PE  : f32r one-hot matvec (sum over n), f32r weighted matvecs
        (output/selected), fp32 transposes + small gate matmuls
"""

import os
import sys

import numpy as np

for _p in ("/root/.axon_site/_ro/trn_rl_repo", "/opt/trn_rl_repo"):
    if os.path.isdir(_p) and _p not in sys.path:
        sys.path.insert(0, _p)

import concourse.bacc as bacc
import concourse.mybir as mybir
import concourse.tile as tile
from concourse import bass_isa
from concourse.bass_utils import run_bass_kernel_spmd
from contextlib import ExitStack

F32 = mybir.dt.float32
F32R = mybir.dt.float32r
AF = mybir.ActivationFunctionType
ALU = mybir.AluOpType

B, W, N, H = 8, 32, 400, 256
TOPK = 10
BN_EPS = 1e-5
NP = 100          # partitions per n-chunk
NC = 4            # n chunks
GW = 8            # windows per group
NGROUPS = W // GW # 4
INV_N = 1.0 / N
INV_SQRT2 = 0.7071067811865476

_CACHED = {}


def _build_nc():
    nc = bacc.Bacc(num_devices=8, debug=False, target_bir_lowering=False)

    xb = nc.dram_tensor("xb", [W, N, H], F32, kind="ExternalInput")
    wt = nc.dram_tensor("wt", [128, 512], F32, kind="ExternalInput")      # W' h-major blocks
    bpp = nc.dram_tensor("bpp", [128, 2], F32, kind="ExternalInput")      # b' per kblk
    watt = nc.dram_tensor("watt", [128, 800], F32, kind="ExternalInput")  # W_attend k-major blocks
    battr = nc.dram_tensor("battr", [GW, 400], F32, kind="ExternalInput") # b_attend replicated
    ident = nc.dram_tensor("ident", [128, 128], F32, kind="ExternalInput")
    sel4 = nc.dram_tensor("sel4", [128, 16], F32, kind="ExternalInput")   # tiled eye(4)
    zeros_in = nc.dram_tensor("zeros_in", [128, 128], F32, kind="ExternalInput")

    o_out01 = nc.dram_tensor("o_out01", [2 * W, H], F32, kind="ExternalOutput")
    o_gate = nc.dram_tensor("o_gate", [W, N], F32, kind="ExternalOutput")

    with tile.TileContext(nc) as tc, ExitStack() as ctx:
        consts = ctx.enter_context(tc.tile_pool(name="consts", bufs=1))
        xf32p = ctx.enter_context(tc.tile_pool(name="xf32", bufs=3))
        xrp = ctx.enter_context(tc.tile_pool(name="xr", bufs=12))
        m1p = ctx.enter_context(tc.tile_pool(name="m1", bufs=2))
        mmp = ctx.enter_context(tc.tile_pool(name="mm", bufs=2))
        mrp = ctx.enter_context(tc.tile_pool(name="mr", bufs=2))
        grp = ctx.enter_context(tc.tile_pool(name="grp", bufs=2))
        ps_sum = ctx.enter_context(tc.tile_pool(name="ps_sum", bufs=2, space="PSUM"))
        ps_misc = ctx.enter_context(tc.tile_pool(name="ps_misc", bufs=1, space="PSUM"))
        ps_wt = ctx.enter_context(tc.tile_pool(name="ps_wt", bufs=1, space="PSUM"))
        ps_o = ctx.enter_context(tc.tile_pool(name="ps_o", bufs=2, space="PSUM"))

        # ---- constants ----
        wt_sb = consts.tile([128, 512], F32)
        nc.sync.dma_start(wt_sb[:], wt[:])
        bpp_sb = consts.tile([128, 2], F32)
        nc.sync.dma_start(bpp_sb[:], bpp[:])
        watt_sb = consts.tile([128, 800], F32)
        nc.sync.dma_start(watt_sb[:], watt[:])
        battr_sb = consts.tile([GW, 400], F32)
        nc.sync.dma_start(battr_sb[:], battr[:])
        ident_sb = consts.tile([128, 128], F32)
        nc.sync.dma_start(ident_sb[:], ident[:])
        sel4_sb = consts.tile([128, 16], F32)
        nc.sync.dma_start(sel4_sb[:], sel4[:])
        sel4r = consts.tile([128, 16], F32R)
        nc.scalar.copy(sel4r[:], sel4_sb[:])
        zeros_sb = consts.tile([128, 128], F32)
        nc.sync.dma_start(zeros_sb[:], zeros_in[:])
        wpad_tiles = []
        for i in range(2):
            wpc = consts.tile([NP, 4, NC, 8], F32R, tag="wpadc", name=f"wpadc_{i}", bufs=2)
            nc.scalar.copy(
                wpc[:],
                zeros_sb[0:NP, :].rearrange("p (a c s) -> p a c s", a=4, c=NC),
            )
            wpad_tiles.append(wpc)

        xr_tiles = []

        for g in range(NGROUPS):
            g0 = g * GW
            psum_sum_h = [
                ps_sum.tile([4, H], F32, tag="psum_sum", name=f"pssum_{g}_{i}")
                for i in range(2)
            ]
            rall_h = [
                grp.tile([4, H], F32, tag="rall_h", name=f"rall_h_{g}_{i}")
                for i in range(2)
            ]
            sum_started = [False, False]
            pmisc = ps_misc.tile([128, 512], F32, tag="pmisc", name=f"pmisc_{g}")
            psum_xrt = pmisc[:, 0:16]
            psum_e = pmisc[:, 16:32]
            psum_g = pmisc[0:GW, 32:432]

            for pp in range(GW // 2):  # pairs within group
                pair = g * (GW // 2) + pp
                # ---- load pair of windows ----
                xf = xf32p.tile([NP, 2048], F32)
                src = xb[2 * pair : 2 * pair + 2].rearrange(
                    "t (c p) h -> p t c h", c=NC, p=NP
                )
                nc.sync.dma_start(
                    xf[:].rearrange("p (t c h) -> p t c h", t=2, c=NC), src
                )

                # ---- round to f32r (Act does 3/4, Pool does 1/4) ----
                xr = xrp.tile([NP, 2048], F32R)
                nc.scalar.copy(xr[:, 0:1536], xf[:, 0:1536])
                nc.gpsimd.tensor_copy(xr[:, 1536:2048], xf[:, 1536:2048])
                xr_tiles.append(xr)

                # ---- cross-chunk max fold (exact fp32) ----
                m1 = m1p.tile([NP, 1024], F32)
                nc.vector.tensor_tensor(
                    out=m1[:].rearrange("p (t q) -> p t q", t=2),
                    in0=xf[:].rearrange("p (t q) -> p t q", t=2)[:, :, 0:512],
                    in1=xf[:].rearrange("p (t q) -> p t q", t=2)[:, :, 512:1024],
                    op=ALU.max,
                )
                mm = mmp.tile([NP, 512], F32)
                nc.vector.tensor_tensor(
                    out=mm[:].rearrange("p (t q) -> p t q", t=2),
                    in0=m1[:].rearrange("p (t q) -> p t q", t=2)[:, :, 0:256],
                    in1=m1[:].rearrange("p (t q) -> p t q", t=2)[:, :, 256:512],
                    op=ALU.max,
                )
                # ---- partition max (replicated across rows) ----
                mr = mrp.tile([NP, 512], F32)
                nc.gpsimd.partition_all_reduce(
                    mr[:], mm[:], channels=NP, reduce_op=bass_isa.ReduceOp.max
                )
                # transpose mr row 0 (the max) into the readout-T psum cols
                for t in range(2):
                    r = 2 * pp + t
                    half, ri = divmod(r, 4)
                    for hb in range(2):
                        col = hb * 8 + half * 4 + ri
                        nc.tensor.matmul(
                            psum_xrt[:, col : col + 1],
                            mr[0:1, t * 256 + hb * 128 : t * 256 + (hb + 1) * 128],
                            ident_sb[0:1, 0:1],
                            is_transpose=True,
                            start=(pp == 0 and t == 0 and hb == 0),
                            stop=True,
                            skip_group_check=True,
                        )

                # ---- f32r one-hot matvec: sum over n into psum row ri ----
                for t in range(2):
                    r = 2 * pp + t
                    half, ri = divmod(r, 4)
                    pt = psum_sum_h[half]
                    for c in range(NC):
                        st = not sum_started[half]
                        sum_started[half] = True
                        nc.tensor.matmul(
                            pt[:, :],
                            sel4r[0:NP, ri * 4 : (ri + 1) * 4],
                            xr[:, t * 1024 + c * 256 : t * 1024 + (c + 1) * 256],
                            start=st,
                            stop=(pp == GW // 2 - 1 and t == 1 and c == NC - 1),
                            skip_group_check=True,
                        )

            # ======== group phase: readout -> gate -> weights ========
            for half in range(2):
                # mean rows = psum_sum / N (scaled PSUM evacuation on Act)
                nc.scalar.activation(
                    rall_h[half][:], psum_sum_h[half][:], AF.Copy, scale=INV_N
                )
                # accumulate mean^T onto the max^T columns already in psum_xrt
                for hb in range(2):
                    nc.tensor.matmul(
                        psum_xrt[:, hb * 8 + half * 4 : hb * 8 + half * 4 + 4],
                        rall_h[half][:, hb * 128 : (hb + 1) * 128],
                        ident_sb[0:4, 0:4],
                        is_transpose=True,
                        start=False,
                        stop=True,
                        skip_group_check=True,
                    )
            xrt_sb = grp.tile([128, 16], F32)
            nc.scalar.copy(xrt_sb[:], psum_xrt[:])

            # e^T = W'^T @ xr^T   [k, w8] (2 k-blocks); col = (kb, half, ri)
            for kb in range(2):
                for hb in range(2):
                    nc.tensor.matmul(
                        psum_e[:, kb * 8 : (kb + 1) * 8],
                        wt_sb[:, hb * 256 + kb * 128 : hb * 256 + (kb + 1) * 128],
                        xrt_sb[:, hb * 8 : (hb + 1) * 8],
                        start=False,
                        stop=(hb == 1),
                        skip_group_check=True,
                    )
            # z = e + b' ; gelu(z) = 0.5 z (1 + erf(z/sqrt2))
            z_sb = grp.tile([128, 16], F32)
            for kb in range(2):
                nc.vector.tensor_scalar(
                    out=z_sb[:, kb * 8 : (kb + 1) * 8],
                    in0=psum_e[:, kb * 8 : (kb + 1) * 8],
                    scalar1=bpp_sb[:, kb : kb + 1],
                    scalar2=None,
                    op0=ALU.add,
                )
            u_sb = grp.tile([128, 16], F32)
            nc.scalar.activation(u_sb[:], z_sb[:], AF.Erf, scale=INV_SQRT2)
            u1_sb = grp.tile([128, 16], F32)
            nc.vector.tensor_scalar(
                out=u1_sb[:], in0=u_sb[:], scalar1=1.0, scalar2=None, op0=ALU.add
            )
            etg_sb = grp.tile([128, 16], F32)
            nc.vector.scalar_tensor_tensor(
                out=etg_sb[:],
                in0=z_sb[:],
                scalar=0.5,
                in1=u1_sb[:],
                op0=ALU.mult,
                op1=ALU.mult,
            )
            # gate logits [w8, n]
            for kb in range(2):
                nc.tensor.matmul(
                    psum_g[:, :],
                    etg_sb[:, kb * 8 : (kb + 1) * 8],
                    watt_sb[:, kb * 400 : (kb + 1) * 400],
                    start=False,
                    stop=(kb == 1),
                    skip_group_check=True,
                )
            logits_sb = grp.tile([GW, 400], F32)
            nc.vector.tensor_tensor(
                out=logits_sb[:], in0=psum_g[:], in1=battr_sb[:], op=ALU.add
            )
            gate_g = grp.tile([GW, 400], F32)
            nc.scalar.activation(gate_g[:], logits_sb[:], AF.Sigmoid)
            nc.gpsimd.dma_start(o_gate[g0 : g0 + GW, :], gate_g[:])

            # ---- topk(10) via hw top-8 x2 + match_replace ----
            zap_g = grp.tile([GW, N], F32)
            wsel_g = grp.tile([GW, N], F32)
            woutv_g = grp.tile([GW, N], F32)
            m8_g = grp.tile([GW, 16], F32)
            nc.vector.tensor_copy(zap_g[:], gate_g[:])
            nc.vector.max(out=m8_g[:, 0:8], in_=zap_g[:])
            nc.vector.match_replace(
                out=zap_g[:], in_to_replace=m8_g[:, 0:8], in_values=zap_g[:],
                imm_value=0.0,
            )
            nc.vector.max(out=m8_g[:, 8:16], in_=zap_g[:])
            nc.vector.memset(m8_g[:, 10:16], 0.0)
            nc.vector.match_replace(
                out=zap_g[:], in_to_replace=m8_g[:, 8:16], in_values=zap_g[:],
                imm_value=0.0,
            )
            nc.vector.tensor_tensor(
                out=wsel_g[:], in0=gate_g[:], in1=zap_g[:], op=ALU.subtract
            )
            nc.vector.tensor_scalar(
                out=woutv_g[:], in0=gate_g[:], scalar1=INV_N, scalar2=None,
                op0=ALU.mult,
            )

            # ---- transpose weights and build zero-padded one-hot lhsT ----
            psum_wt = ps_wt.tile([NP, NC, 2, GW], F32)
            for c in range(NC):
                nc.tensor.matmul(
                    psum_wt[:, c, 0, :],
                    woutv_g[:, c * NP : (c + 1) * NP],
                    ident_sb[0:GW, 0:GW],
                    is_transpose=True,
                    start=(c == 0),
                    stop=True,
                    skip_group_check=True,
                )
                nc.tensor.matmul(
                    psum_wt[:, c, 1, :],
                    wsel_g[:, c * NP : (c + 1) * NP],
                    ident_sb[0:GW, 0:GW],
                    is_transpose=True,
                    start=False,
                    stop=True,
                    skip_group_check=True,
                )
            # wpad[half]: [NP, (ri, c, 8)] f32r, zero except cols (2ri, 2ri+1)
            # (persistent pre-zeroed tiles; only the one-hot cols are rewritten)
            wpads = wpad_tiles
            for half in range(2):
                wpad = wpads[half]
                for ri in range(4):
                    if ri % 2 == 0:
                        nc.scalar.copy(
                            wpad[:, ri, :, 2 * ri : 2 * ri + 2],
                            psum_wt[:, :, :, half * 4 + ri],
                        )
                    else:
                        nc.vector.tensor_scalar(
                            out=wpad[:, ri, :, 2 * ri : 2 * ri + 2],
                            in0=psum_wt[:, :, :, half * 4 + ri],
                            scalar1=0.0,
                            scalar2=None,
                            op0=ALU.add,
                        )

            # ---- sweep-2: weighted matvecs (f32r, one-hot padded rows) ----
            for half in range(2):
                psum_o = ps_o.tile([GW, H], F32)
                wpad = wpads[half]
                first = True
                for ri in range(4):
                    w = g0 + half * 4 + ri
                    pair, t = divmod(w, 2)
                    xr = xr_tiles[pair]
                    for c in range(NC):
                        nc.tensor.matmul(
                            psum_o[:, :],
                            wpad[:, ri, c, :],
                            xr[:, t * 1024 + c * 256 : t * 1024 + (c + 1) * 256],
                            start=first,
                            stop=(ri == 3 and c == NC - 1),
                            skip_group_check=True,
                        )
                        first = False
                out_sb = grp.tile([GW, H], F32, tag="out_sb",
                                  name=f"out_sb_{g}_{half}")
                nc.scalar.copy(out_sb[:], psum_o[:])
                w0 = g0 + half * 4
                nc.gpsimd.dma_start(o_out01[2 * w0 : 2 * w0 + 8, :], out_sb[:])

    nc.finalize()
    return nc


def _host_prep(W_embed, b_embed, bn_gamma, bn_beta, bn_mean, bn_var, W_attend, b_attend):
    s = (bn_gamma / np.sqrt(bn_var + BN_EPS)).astype(np.float32)
    Wp = (W_embed * s[None, :]).astype(np.float32)          # [H, H]
    bp = (b_embed * s + bn_beta - bn_mean * s).astype(np.float32)  # [H]
    wt = np.ascontiguousarray(
        Wp.reshape(2, 128, 256).transpose(1, 0, 2).reshape(128, 512)
    )
    bpp = np.ascontiguousarray(bp.reshape(2, 128).T)        # [128, 2]
    watt = np.ascontiguousarray(
        W_attend.reshape(2, 128, 400).transpose(1, 0, 2).reshape(128, 800)
    )
    battr = np.ascontiguousarray(np.tile(b_attend[None, :], (GW, 1)).astype(np.float32))
    ident = np.eye(128, dtype=np.float32)
    sel4 = np.ascontiguousarray(
        np.tile(np.eye(4, dtype=np.float32).reshape(1, 16), (128, 1))
    )
    return wt, bpp, watt, battr, ident, sel4


def kernel(x, W_embed, b_embed, bn_gamma, bn_beta, bn_mean, bn_var, W_attend, b_attend):
    x = np.asarray(x, dtype=np.float32)
    wt, bpp, watt, battr, ident, sel4 = _host_prep(
        np.asarray(W_embed, np.float32), np.asarray(b_embed, np.float32),
        np.asarray(bn_gamma, np.float32), np.asarray(bn_beta, np.float32),
        np.asarray(bn_mean, np.float32), np.asarray(bn_var, np.float32),
        np.asarray(W_attend, np.float32), np.asarray(b_attend, np.float32),
    )

    if "nc" not in _CACHED:
        _CACHED["nc"] = _build_nc()
    nc = _CACHED["nc"]

    in_maps = []
    for b in range(B):
        in_maps.append({
            "xb": np.ascontiguousarray(x[b]),
            "wt": wt, "bpp": bpp, "watt": watt,
            "battr": battr, "ident": ident, "sel4": sel4,
            "zeros_in": np.zeros((128, 128), np.float32),
        })
    res = run_bass_kernel_spmd(nc, in_maps, list(range(B)))
    out01 = np.stack([r["o_out01"] for r in res.results], axis=0)  # [B, 64, H]
    out01 = out01.reshape(B, W, 2, H)
    output = np.ascontiguousarray(out01[:, :, 0, :])               # [B, W, H]
    selected = np.ascontiguousarray(out01[:, :, 1, :])             # [B, W, H]
    gate3 = np.stack([r["o_gate"] for r in res.results], axis=1)   # [W, B, N]
    return output, selected, gate3


# revision 29
# speedup vs baseline: 1.0425x; 1.0425x over previous
"""Trainium2 Bass kernel for nn_BARO_89318139887969 (topk_masking).

Computation per (b, w):
  readout = mean_n(x) + max_n(x)                      [h]
  e = gelu(BN(readout @ W_embed + b_embed))           [h]   (BN folded into W', b')
  gate = sigmoid(e @ W_attend + b_attend)             [n]
  output = mean_n(x * gate)                           [h]
  selected = sum over top-10-gate rows of (x * gate)  [h]
  out3 = gate transposed to [w, b, n]

Sharding: data-parallel over batch b across 8 cores (1 batch each).
Per-core layout: x_b [32, 400, 256] processed in 16 pairs of windows,
each pair's x in SBUF as [100, (t=2, c=4, h=256)] (n split into 4 chunks
of 100 partitions).

Compute engines can neither shift nor stride partitions, and fp32/f32r
matmuls reject tile_position; so per-window PSUM rows are produced via
zero-padded one-hot weight columns (zeros accumulate harmlessly into the
other rows), keeping everything at base partition 0.

Engine split (per window):
  DMA : x pair load (contiguous 1KB rows)
  Act : fp32 -> f32r rounding cast (major share), PSUM evacuations, erf/sigmoid
  DVE : cross-chunk max fold (exact fp32), topk (hw top-8 + match_replace)
  Pool: partition_all_reduce max over n, cast share, output DMA issue
  PE  : f32r one-hot matvec (sum over n), f32r weighted matvecs
        (output/selected), fp32 transposes + small gate matmuls
"""

import os
import sys

import numpy as np

for _p in ("/root/.axon_site/_ro/trn_rl_repo", "/opt/trn_rl_repo"):
    if os.path.isdir(_p) and _p not in sys.path:
        sys.path.insert(0, _p)

import concourse.bacc as bacc
import concourse.mybir as mybir
import concourse.tile as tile
from concourse import bass_isa
from concourse.bass_utils import run_bass_kernel_spmd
from contextlib import ExitStack
from concourse.bass_types import AP

F32 = mybir.dt.float32
F32R = mybir.dt.float32r
AF = mybir.ActivationFunctionType
ALU = mybir.AluOpType

B, W, N, H = 8, 32, 400, 256
TOPK = 10
BN_EPS = 1e-5
NP = 100          # partitions per n-chunk
NC = 4            # n chunks
GW = 8            # windows per group
NGROUPS = W // GW # 4
INV_N = 1.0 / N
INV_SQRT2 = 0.7071067811865476

_CACHED = {}


def _build_nc():
    nc = bacc.Bacc(num_devices=8, debug=False, target_bir_lowering=False)

    xb = nc.dram_tensor("xb", [W, N, H], F32, kind="ExternalInput")
    wt = nc.dram_tensor("wt", [128, 512], F32, kind="ExternalInput")      # W' h-major blocks
    bpp = nc.dram_tensor("bpp", [128, 2], F32, kind="ExternalInput")      # b' per kblk
    watt = nc.dram_tensor("watt", [128, 800], F32, kind="ExternalInput")  # W_attend k-major blocks
    battr = nc.dram_tensor("battr", [GW, 400], F32, kind="ExternalInput") # b_attend replicated
    ident = nc.dram_tensor("ident", [128, 128], F32, kind="ExternalInput")
    sel8 = nc.dram_tensor("sel8", [128, 64], F32, kind="ExternalInput")   # tiled eye(8)
    zeros_in = nc.dram_tensor("zeros_in", [128, 128], F32, kind="ExternalInput")

    o_out01 = nc.dram_tensor("o_out01", [2 * W, H], F32, kind="ExternalOutput")
    o_gate = nc.dram_tensor("o_gate", [W, N], F32, kind="ExternalOutput")

    with tile.TileContext(nc) as tc, ExitStack() as ctx:
        consts = ctx.enter_context(tc.tile_pool(name="consts", bufs=1))
        xf32p = ctx.enter_context(tc.tile_pool(name="xf32", bufs=4))
        xrp = ctx.enter_context(tc.tile_pool(name="xr", bufs=12))
        m1p = ctx.enter_context(tc.tile_pool(name="m1", bufs=2))
        mmp = ctx.enter_context(tc.tile_pool(name="mm", bufs=2))
        mrp = ctx.enter_context(tc.tile_pool(name="mr", bufs=2))
        grp = ctx.enter_context(tc.tile_pool(name="grp", bufs=2))
        ps_sum = ctx.enter_context(tc.tile_pool(name="ps_sum", bufs=2, space="PSUM"))
        ps_misc = ctx.enter_context(tc.tile_pool(name="ps_misc", bufs=2, space="PSUM"))
        ps_wt = ctx.enter_context(tc.tile_pool(name="ps_wt", bufs=2, space="PSUM"))
        ps_o = ctx.enter_context(tc.tile_pool(name="ps_o", bufs=2, space="PSUM"))

        # ---- constants ----
        wt_sb = consts.tile([128, 512], F32)
        nc.sync.dma_start(wt_sb[:], wt[:])
        bpp_sb = consts.tile([128, 2], F32)
        nc.sync.dma_start(bpp_sb[:], bpp[:])
        watt_sb = consts.tile([128, 800], F32)
        nc.sync.dma_start(watt_sb[:], watt[:])
        battr_sb = consts.tile([GW, 400], F32)
        nc.sync.dma_start(battr_sb[:], battr[:])
        ident_sb = consts.tile([128, 128], F32)
        nc.sync.dma_start(ident_sb[:], ident[:])
        sel8_sb = consts.tile([128, 64], F32)
        nc.sync.dma_start(sel8_sb[:], sel8[:])
        sel8r = consts.tile([128, 64], F32R)
        nc.scalar.copy(sel8r[:], sel8_sb[:])
        zeros_sb = consts.tile([128, 128], F32)
        nc.sync.dma_start(zeros_sb[:], zeros_in[:])
        wpad_tiles = []
        for i in range(4):
            wpc = consts.tile([NP, 4, NC, 8], F32R, tag="wpadc", name=f"wpadc_{i}", bufs=4)
            nc.scalar.copy(
                wpc[:],
                zeros_sb[0:NP, :].rearrange("p (a c s) -> p a c s", a=4, c=NC),
            )
            wpad_tiles.append(wpc)

        xr_tiles = []
        gstate = {}

        def emit_pairs(g, phase_of=None):
            g0 = g * GW
            psum_sum = ps_sum.tile([GW, 512], F32, tag="psum_sum", name=f"pssum_{g}")
            staging8 = grp.tile([GW, H], F32, tag="staging8", name=f"staging8_{g}")
            gstate[g] = (psum_sum, staging8)
            sum_first = True
            mm2 = None

            for pp in range(GW // 2):
                pair = g * (GW // 2) + pp
                xf = xf32p.tile([NP, 2048], F32, tag="xf", name=f"xf_{pair}")
                src = xb[2 * pair : 2 * pair + 2].rearrange(
                    "t (c p) h -> p t c h", c=NC, p=NP
                )
                dst = xf[:].rearrange("p (t c h) -> p t c h", t=2, c=NC)
                eng = (nc.sync, nc.scalar, nc.gpsimd)[pair % 3]
                eng.dma_start(dst[:, 0, :, :], src[:, 0, :, :])
                eng2 = (nc.scalar, nc.gpsimd, nc.sync)[pair % 3]
                eng2.dma_start(dst[:, 1, :, :], src[:, 1, :, :])

                xr = xrp.tile([NP, 2048], F32R, tag="xr", name=f"xr_{pair}")
                nc.scalar.copy(xr[:, :], xf[:, :])
                xr_tiles.append(xr)

                m1 = m1p.tile([NP, 1024], F32, tag="m1", name=f"m1_{pair}")
                nc.vector.tensor_tensor(
                    out=m1[:].rearrange("p (t q) -> p t q", t=2),
                    in0=xf[:].rearrange("p (t q) -> p t q", t=2)[:, :, 0:512],
                    in1=xf[:].rearrange("p (t q) -> p t q", t=2)[:, :, 512:1024],
                    op=ALU.max,
                )
                if pp % 2 == 0:
                    mm2 = mmp.tile([NP, 1024], F32, tag="mm2", name=f"mm2_{g}_{pp}")
                nc.vector.tensor_tensor(
                    out=mm2[:, pp % 2 * 512 : pp % 2 * 512 + 512].rearrange(
                        "p (t q) -> p t q", t=2
                    ),
                    in0=m1[:].rearrange("p (t q) -> p t q", t=2)[:, :, 0:256],
                    in1=m1[:].rearrange("p (t q) -> p t q", t=2)[:, :, 256:512],
                    op=ALU.max,
                )
                if pp % 2 == 1:
                    mr = mrp.tile([NP, 1024], F32, tag="mr", name=f"mr_{g}_{pp}")
                    nc.gpsimd.partition_all_reduce(
                        mr[:], mm2[:], channels=NP, reduce_op=bass_isa.ReduceOp.max
                    )
                    r0 = 2 * (pp - 1)
                    nc.gpsimd.dma_start(
                        staging8[r0 : r0 + 4, :],
                        mr[0:1, :].rearrange("p (r h) -> p r h", r=4),
                    )
                if pp == 1 and phase_of is not None:
                    emit_phase(phase_of)

                for t in range(2):
                    r = 2 * pp + t
                    for cp in range(2):
                        nc.tensor.matmul(
                            psum_sum[:, :],
                            sel8r[0:NP, r * 8 : (r + 1) * 8],
                            xr[:, t * 1024 + cp * 512 : t * 1024 + (cp + 1) * 512],
                            start=sum_first,
                            stop=(pp == GW // 2 - 1 and t == 1 and cp == 1),
                            skip_group_check=True,
                        )
                        sum_first = False

        def emit_phase(g):
            g0 = g * GW
            psum_sum, staging8 = gstate.pop(g)
            pmisc = ps_misc.tile([128, 512], F32, tag="pmisc", name=f"pmisc_{g}")
            psum_xrt = pmisc[:, 0:16]
            psum_e = pmisc[:, 16:32]
            psum_g = pmisc[0:GW, 32:432]

            for hb in range(2):
                nc.tensor.matmul(
                    psum_xrt[:, hb * 8 : (hb + 1) * 8],
                    staging8[:, hb * 128 : (hb + 1) * 128],
                    ident_sb[0:GW, 0:GW],
                    is_transpose=True,
                    start=(hb == 0),
                    stop=True,
                    skip_group_check=True,
                )
            rsum8 = grp.tile([GW, 512], F32, tag="rsum8", name=f"rsum8_{g}")
            nc.scalar.activation(rsum8[:], psum_sum[:], AF.Copy, scale=INV_N)
            rall8 = grp.tile([GW, H], F32, tag="rall8", name=f"rall8_{g}")
            nc.vector.tensor_tensor(
                out=rall8[:], in0=rsum8[:, 0:256], in1=rsum8[:, 256:512], op=ALU.add
            )
            for hb in range(2):
                nc.tensor.matmul(
                    psum_xrt[:, hb * 8 : (hb + 1) * 8],
                    rall8[:, hb * 128 : (hb + 1) * 128],
                    ident_sb[0:GW, 0:GW],
                    is_transpose=True,
                    start=False,
                    stop=True,
                    skip_group_check=True,
                )
            xrt_sb = grp.tile([128, 16], F32, tag="xrt_sb", name=f"xrt_{g}")
            nc.scalar.copy(xrt_sb[:], psum_xrt[:])

            for kb in range(2):
                for hb in range(2):
                    nc.tensor.matmul(
                        psum_e[:, kb * 8 : (kb + 1) * 8],
                        wt_sb[:, hb * 256 + kb * 128 : hb * 256 + (kb + 1) * 128],
                        xrt_sb[:, hb * 8 : (hb + 1) * 8],
                        start=False,
                        stop=(hb == 1),
                        skip_group_check=True,
                    )
            z_sb = grp.tile([128, 16], F32, tag="z_sb", name=f"z_{g}")
            for kb in range(2):
                nc.vector.tensor_scalar(
                    out=z_sb[:, kb * 8 : (kb + 1) * 8],
                    in0=psum_e[:, kb * 8 : (kb + 1) * 8],
                    scalar1=bpp_sb[:, kb : kb + 1],
                    scalar2=None,
                    op0=ALU.add,
                )
            u_sb = grp.tile([128, 16], F32, tag="u_sb", name=f"u_{g}")
            nc.scalar.activation(u_sb[:], z_sb[:], AF.Erf, scale=INV_SQRT2)
            etg_sb = grp.tile([128, 16], F32, tag="etg_sb", name=f"etg_{g}")
            nc.vector.scalar_tensor_tensor(
                out=etg_sb[:],
                in0=u_sb[:],
                scalar=1.0,
                in1=z_sb[:],
                op0=ALU.add,
                op1=ALU.mult,
            )
            for kb in range(2):
                nc.tensor.matmul(
                    psum_g[:, :],
                    etg_sb[:, kb * 8 : (kb + 1) * 8],
                    watt_sb[:, kb * 400 : (kb + 1) * 400],
                    start=False,
                    stop=(kb == 1),
                    skip_group_check=True,
                )
            logits_sb = grp.tile([GW, 400], F32, tag="logits", name=f"logits_{g}")
            nc.vector.tensor_tensor(
                out=logits_sb[:], in0=psum_g[:], in1=battr_sb[:], op=ALU.add
            )
            gate_g = grp.tile([GW, 400], F32, tag="gate_g", name=f"gate_{g}")
            nc.scalar.activation(gate_g[:], logits_sb[:], AF.Sigmoid)
            nc.gpsimd.dma_start(o_gate[g0 : g0 + GW, :], gate_g[:])

            zap_g = grp.tile([GW, N], F32, tag="zap_g", name=f"zap_{g}")
            wsel_g = grp.tile([GW, N], F32, tag="wsel_g", name=f"wsel_{g}")
            woutv_g = grp.tile([GW, N], F32, tag="woutv_g", name=f"woutv_{g}")
            m8_g = grp.tile([GW, 16], F32, tag="m8_g", name=f"m8_{g}")
            nc.vector.max(out=m8_g[:, 0:8], in_=gate_g[:])
            nc.vector.match_replace(
                out=zap_g[:], in_to_replace=m8_g[:, 0:8], in_values=gate_g[:],
                imm_value=0.0,
            )
            nc.vector.max(out=m8_g[:, 8:16], in_=zap_g[:])
            nc.vector.memset(m8_g[:, 10:16], 0.0)
            nc.vector.match_replace(
                out=zap_g[:], in_to_replace=m8_g[:, 8:16], in_values=zap_g[:],
                imm_value=0.0,
            )
            nc.vector.tensor_tensor(
                out=wsel_g[:], in0=gate_g[:], in1=zap_g[:], op=ALU.subtract
            )
            nc.vector.tensor_scalar(
                out=woutv_g[:], in0=gate_g[:], scalar1=INV_N, scalar2=None,
                op0=ALU.mult,
            )

            psum_wt = ps_wt.tile([NP, NC, 2, GW], F32, tag="psum_wt", name=f"pswt_{g}")
            for c in range(NC):
                nc.tensor.matmul(
                    psum_wt[:, c, 0, :],
                    woutv_g[:, c * NP : (c + 1) * NP],
                    ident_sb[0:GW, 0:GW],
                    is_transpose=True,
                    start=(c == 0),
                    stop=True,
                    skip_group_check=True,
                )
                nc.tensor.matmul(
                    psum_wt[:, c, 1, :],
                    wsel_g[:, c * NP : (c + 1) * NP],
                    ident_sb[0:GW, 0:GW],
                    is_transpose=True,
                    start=False,
                    stop=True,
                    skip_group_check=True,
                )
            for half in range(2):
                wpad = wpad_tiles[(g % 2) * 2 + half]
                for ri in range(4):
                    if ri % 2 == 0:
                        nc.scalar.copy(
                            wpad[:, ri, :, 2 * ri : 2 * ri + 2],
                            psum_wt[:, :, :, half * 4 + ri],
                        )
                    else:
                        nc.vector.tensor_scalar(
                            out=wpad[:, ri, :, 2 * ri : 2 * ri + 2],
                            in0=psum_wt[:, :, :, half * 4 + ri],
                            scalar1=0.0,
                            scalar2=None,
                            op0=ALU.add,
                        )

            for half in range(2):
                psum_o = ps_o.tile([GW, H], F32, tag="psum_o", name=f"pso_{g}_{half}")
                wpad = wpad_tiles[(g % 2) * 2 + half]
                first = True
                for ri in range(4):
                    w = g0 + half * 4 + ri
                    pair, t = divmod(w, 2)
                    xr = xr_tiles[pair]
                    for c in range(NC):
                        nc.tensor.matmul(
                            psum_o[:, :],
                            wpad[:, ri, c, :],
                            xr[:, t * 1024 + c * 256 : t * 1024 + (c + 1) * 256],
                            start=first,
                            stop=(ri == 3 and c == NC - 1),
                            skip_group_check=True,
                        )
                        first = False
                out_sb = grp.tile([GW, H], F32, tag="out_sb",
                                  name=f"out_sb_{g}_{half}")
                nc.scalar.copy(out_sb[:], psum_o[:])
                w0 = g0 + half * 4
                nc.gpsimd.dma_start(o_out01[2 * w0 : 2 * w0 + 8, :], out_sb[:])

        # software pipeline: group g-1's phase emitted mid-way through group g
        for g in range(NGROUPS):
            emit_pairs(g, phase_of=g - 1 if g > 0 else None)
        emit_phase(NGROUPS - 1)

    nc.finalize()
    return nc


def _host_prep(W_embed, b_embed, bn_gamma, bn_beta, bn_mean, bn_var, W_attend, b_attend):
    s = (bn_gamma / np.sqrt(bn_var + BN_EPS)).astype(np.float32)
    Wp = (W_embed * s[None, :]).astype(np.float32)          # [H, H]
    bp = (b_embed * s + bn_beta - bn_mean * s).astype(np.float32)  # [H]
    wt = np.ascontiguousarray(
        Wp.reshape(2, 128, 256).transpose(1, 0, 2).reshape(128, 512)
    )
    bpp = np.ascontiguousarray(bp.reshape(2, 128).T)        # [128, 2]
    watt = np.ascontiguousarray(
        (0.5 * W_attend).reshape(2, 128, 400).transpose(1, 0, 2).reshape(128, 800)
    )
    battr = np.ascontiguousarray(np.tile(b_attend[None, :], (GW, 1)).astype(np.float32))
    ident = np.eye(128, dtype=np.float32)
    sel4 = np.ascontiguousarray(
        np.tile(np.eye(4, dtype=np.float32).reshape(1, 16), (128, 1))
    )
    return wt, bpp, watt, battr, ident, sel4


def kernel(x, W_embed, b_embed, bn_gamma, bn_beta, bn_mean, bn_var, W_attend, b_attend):
    x = np.asarray(x, dtype=np.float32)
    wt, bpp, watt, battr, ident, sel4 = _host_prep(
        np.asarray(W_embed, np.float32), np.asarray(b_embed, np.float32),
        np.asarray(bn_gamma, np.float32), np.asarray(bn_beta, np.float32),
        np.asarray(bn_mean, np.float32), np.asarray(bn_var, np.float32),
        np.asarray(W_attend, np.float32), np.asarray(b_attend, np.float32),
    )

    if "nc" not in _CACHED:
        _CACHED["nc"] = _build_nc()
    nc = _CACHED["nc"]

    in_maps = []
    for b in range(B):
        in_maps.append({
            "xb": np.ascontiguousarray(x[b]),
            "wt": wt, "bpp": bpp, "watt": watt,
            "battr": battr, "ident": ident, "sel4": sel4,
            "zeros_in": np.zeros((128, 128), np.float32),
        })
    res = run_bass_kernel_spmd(nc, in_maps, list(range(B)))
    out01 = np.stack([r["o_out01"] for r in res.results], axis=0)  # [B, 64, H]
    out01 = out01.reshape(B, W, 2, H)
    output = np.ascontiguousarray(out01[:, :, 0, :])               # [B, W, H]
    selected = np.ascontiguousarray(out01[:, :, 1, :])             # [B, W, H]
    gate3 = np.stack([r["o_gate"] for r in res.results], axis=1)   # [W, B, N]
    return output, selected, gate3


# revision 30
# speedup vs baseline: 1.1457x; 1.0990x over previous
"""Trainium2 Bass kernel for nn_BARO_89318139887969 (topk_masking).

Computation per (b, w):
  readout = mean_n(x) + max_n(x)                      [h]
  e = gelu(BN(readout @ W_embed + b_embed))           [h]   (BN folded into W', b')
  gate = sigmoid(e @ W_attend + b_attend)             [n]
  output = mean_n(x * gate)                           [h]
  selected = sum over top-10-gate rows of (x * gate)  [h]
  out3 = gate transposed to [w, b, n]

Sharding: data-parallel over batch b across 8 cores (1 batch each).
Per-core layout: x_b [32, 400, 256] processed in 16 pairs of windows,
each pair's x in SBUF as [100, (t=2, c=4, h=256)] (n split into 4 chunks
of 100 partitions).

Compute engines can neither shift nor stride partitions, and fp32/f32r
matmuls reject tile_position; so per-window PSUM rows are produced via
zero-padded one-hot weight columns (zeros accumulate harmlessly into the
other rows), keeping everything at base partition 0.

Engine split (per window):
  DMA : x pair load (contiguous 1KB rows)
  Act : fp32 -> f32r rounding cast (major share), PSUM evacuations, erf/sigmoid
  DVE : cross-chunk max fold (exact fp32), topk (hw top-8 + match_replace)
  Pool: partition_all_reduce max over n, cast share, output DMA issue
  PE  : f32r one-hot matvec (sum over n), f32r weighted matvecs
        (output/selected), fp32 transposes + small gate matmuls
"""

import os
import sys

import numpy as np

for _p in ("/root/.axon_site/_ro/trn_rl_repo", "/opt/trn_rl_repo"):
    if os.path.isdir(_p) and _p not in sys.path:
        sys.path.insert(0, _p)

import concourse.bacc as bacc
import concourse.mybir as mybir
import concourse.tile as tile
from concourse import bass_isa
from concourse.bass_utils import run_bass_kernel_spmd
from contextlib import ExitStack
from concourse.bass_types import AP

F32 = mybir.dt.float32
F32R = mybir.dt.float32r
AF = mybir.ActivationFunctionType
ALU = mybir.AluOpType

B, W, N, H = 8, 32, 400, 256
TOPK = 10
BN_EPS = 1e-5
NP = 100          # partitions per n-chunk
NC = 4            # n chunks
GW = 8            # windows per group
NGROUPS = W // GW # 4
INV_N = 1.0 / N
INV_SQRT2 = 0.7071067811865476

_CACHED = {}


def _build_nc():
    nc = bacc.Bacc(num_devices=8, debug=False, target_bir_lowering=False)

    xb = nc.dram_tensor("xb", [W, N, H], F32, kind="ExternalInput")
    wt = nc.dram_tensor("wt", [128, 512], F32, kind="ExternalInput")      # W' h-major blocks
    bpp = nc.dram_tensor("bpp", [128, 2], F32, kind="ExternalInput")      # b' per kblk
    watt = nc.dram_tensor("watt", [128, 800], F32, kind="ExternalInput")  # W_attend k-major blocks
    battr = nc.dram_tensor("battr", [GW, 400], F32, kind="ExternalInput") # b_attend replicated
    ident = nc.dram_tensor("ident", [128, 128], F32, kind="ExternalInput")
    sel8 = nc.dram_tensor("sel8", [128, 64], F32, kind="ExternalInput")   # tiled eye(8)
    zeros_in = nc.dram_tensor("zeros_in", [128, 128], F32, kind="ExternalInput")

    o_out01 = nc.dram_tensor("o_out01", [2 * W, H], F32, kind="ExternalOutput")
    o_gate = nc.dram_tensor("o_gate", [W, N], F32, kind="ExternalOutput")

    with tile.TileContext(nc) as tc, ExitStack() as ctx:
        consts = ctx.enter_context(tc.tile_pool(name="consts", bufs=1))
        xf32p = ctx.enter_context(tc.tile_pool(name="xf32", bufs=3))
        xrp = ctx.enter_context(tc.tile_pool(name="xr", bufs=13))
        m1p = ctx.enter_context(tc.tile_pool(name="m1", bufs=2))
        mmp = ctx.enter_context(tc.tile_pool(name="mm", bufs=2))
        mrp = ctx.enter_context(tc.tile_pool(name="mr", bufs=2))
        grp = ctx.enter_context(tc.tile_pool(name="grp", bufs=2))
        ps_sum = ctx.enter_context(tc.tile_pool(name="ps_sum", bufs=2, space="PSUM"))
        ps_misc = ctx.enter_context(tc.tile_pool(name="ps_misc", bufs=2, space="PSUM"))
        ps_wt = ctx.enter_context(tc.tile_pool(name="ps_wt", bufs=2, space="PSUM"))
        ps_o = ctx.enter_context(tc.tile_pool(name="ps_o", bufs=2, space="PSUM"))

        # ---- constants ----
        wt_sb = consts.tile([128, 512], F32)
        nc.sync.dma_start(wt_sb[:], wt[:])
        bpp_sb = consts.tile([128, 2], F32)
        nc.sync.dma_start(bpp_sb[:], bpp[:])
        watt_sb = consts.tile([128, 800], F32)
        nc.sync.dma_start(watt_sb[:], watt[:])
        battr_sb = consts.tile([GW, 400], F32)
        nc.sync.dma_start(battr_sb[:], battr[:])
        ident_sb = consts.tile([128, 128], F32)
        nc.sync.dma_start(ident_sb[:], ident[:])
        sel8_sb = consts.tile([128, 64], F32)
        nc.sync.dma_start(sel8_sb[:], sel8[:])
        sel8r = consts.tile([128, 64], F32R)
        nc.scalar.copy(sel8r[:], sel8_sb[:])
        zeros_sb = consts.tile([128, 128], F32)
        nc.sync.dma_start(zeros_sb[:], zeros_in[:])
        wpad_tiles = []
        for i in range(4):
            wpc = consts.tile([NP, 4, NC, 8], F32R, tag="wpadc", name=f"wpadc_{i}", bufs=4)
            nc.scalar.copy(
                wpc[:],
                zeros_sb[0:NP, :].rearrange("p (a c s) -> p a c s", a=4, c=NC),
            )
            wpad_tiles.append(wpc)

        xr_tiles = []
        gstate = {}

        def emit_pairs(g, phase_of=None):
            g0 = g * GW
            psum_sum = ps_sum.tile([GW, 512], F32, tag="psum_sum", name=f"pssum_{g}")
            staging8 = grp.tile([GW, H], F32, tag="staging8", name=f"staging8_{g}")
            gstate[g] = (psum_sum, staging8)
            sum_first = True
            mm2 = None

            for pp in range(GW // 2):
                pair = g * (GW // 2) + pp
                xf = xf32p.tile([NP, 2048], F32, tag="xf", name=f"xf_{pair}")
                src = xb[2 * pair : 2 * pair + 2].rearrange(
                    "t (c p) h -> p t c h", c=NC, p=NP
                )
                dst = xf[:].rearrange("p (t c h) -> p t c h", t=2, c=NC)
                eng = (nc.sync, nc.scalar, nc.gpsimd)[pair % 3]
                eng.dma_start(dst[:, 0, :, :], src[:, 0, :, :])
                eng2 = (nc.scalar, nc.gpsimd, nc.sync)[pair % 3]
                eng2.dma_start(dst[:, 1, :, :], src[:, 1, :, :])

                xr = xrp.tile([NP, 2048], F32R, tag="xr", name=f"xr_{pair}")
                nc.scalar.copy(xr[:, :], xf[:, :])
                xr_tiles.append(xr)

                m1 = m1p.tile([NP, 1024], F32, tag="m1", name=f"m1_{pair}")
                nc.vector.tensor_tensor(
                    out=m1[:].rearrange("p (t q) -> p t q", t=2),
                    in0=xf[:].rearrange("p (t q) -> p t q", t=2)[:, :, 0:512],
                    in1=xf[:].rearrange("p (t q) -> p t q", t=2)[:, :, 512:1024],
                    op=ALU.max,
                )
                if pp % 2 == 0:
                    mm2 = mmp.tile([NP, 1024], F32, tag="mm2", name=f"mm2_{g}_{pp}")
                nc.vector.tensor_tensor(
                    out=mm2[:, pp % 2 * 512 : pp % 2 * 512 + 512].rearrange(
                        "p (t q) -> p t q", t=2
                    ),
                    in0=m1[:].rearrange("p (t q) -> p t q", t=2)[:, :, 0:256],
                    in1=m1[:].rearrange("p (t q) -> p t q", t=2)[:, :, 256:512],
                    op=ALU.max,
                )
                if pp % 2 == 1:
                    mr = mrp.tile([NP, 1024], F32, tag="mr", name=f"mr_{g}_{pp}")
                    nc.gpsimd.partition_all_reduce(
                        mr[:], mm2[:], channels=NP, reduce_op=bass_isa.ReduceOp.max
                    )
                    r0 = 2 * (pp - 1)
                    nc.sync.dma_start(
                        staging8[r0 : r0 + 4, :],
                        mr[0:1, :].rearrange("p (r h) -> p r h", r=4),
                    )
                if pp == 1 and phase_of is not None:
                    emit_phase(phase_of)

                for t in range(2):
                    r = 2 * pp + t
                    for cp in range(2):
                        nc.tensor.matmul(
                            psum_sum[:, :],
                            sel8r[0:NP, r * 8 : (r + 1) * 8],
                            xr[:, t * 1024 + cp * 512 : t * 1024 + (cp + 1) * 512],
                            start=sum_first,
                            stop=(pp == GW // 2 - 1 and t == 1 and cp == 1),
                            skip_group_check=True,
                        )
                        sum_first = False

        def emit_phase(g):
            g0 = g * GW
            psum_sum, staging8 = gstate.pop(g)
            pmisc = ps_misc.tile([128, 512], F32, tag="pmisc", name=f"pmisc_{g}")
            psum_xrt = pmisc[:, 0:16]
            psum_e = pmisc[:, 16:32]
            psum_g = pmisc[0:GW, 32:432]

            for hb in range(2):
                nc.tensor.matmul(
                    psum_xrt[:, hb * 8 : (hb + 1) * 8],
                    staging8[:, hb * 128 : (hb + 1) * 128],
                    ident_sb[0:GW, 0:GW],
                    is_transpose=True,
                    start=(hb == 0),
                    stop=True,
                    skip_group_check=True,
                )
            rsum8 = grp.tile([GW, 512], F32, tag="rsum8", name=f"rsum8_{g}")
            nc.scalar.activation(rsum8[:], psum_sum[:], AF.Copy, scale=INV_N)
            rall8 = grp.tile([GW, H], F32, tag="rall8", name=f"rall8_{g}")
            nc.vector.tensor_tensor(
                out=rall8[:], in0=rsum8[:, 0:256], in1=rsum8[:, 256:512], op=ALU.add
            )
            for hb in range(2):
                nc.tensor.matmul(
                    psum_xrt[:, hb * 8 : (hb + 1) * 8],
                    rall8[:, hb * 128 : (hb + 1) * 128],
                    ident_sb[0:GW, 0:GW],
                    is_transpose=True,
                    start=False,
                    stop=True,
                    skip_group_check=True,
                )
            xrt_sb = grp.tile([128, 16], F32, tag="xrt_sb", name=f"xrt_{g}")
            nc.scalar.copy(xrt_sb[:], psum_xrt[:])

            for kb in range(2):
                for hb in range(2):
                    nc.tensor.matmul(
                        psum_e[:, kb * 8 : (kb + 1) * 8],
                        wt_sb[:, hb * 256 + kb * 128 : hb * 256 + (kb + 1) * 128],
                        xrt_sb[:, hb * 8 : (hb + 1) * 8],
                        start=False,
                        stop=(hb == 1),
                        skip_group_check=True,
                    )
            z_sb = grp.tile([128, 16], F32, tag="z_sb", name=f"z_{g}")
            for kb in range(2):
                nc.vector.tensor_scalar(
                    out=z_sb[:, kb * 8 : (kb + 1) * 8],
                    in0=psum_e[:, kb * 8 : (kb + 1) * 8],
                    scalar1=bpp_sb[:, kb : kb + 1],
                    scalar2=None,
                    op0=ALU.add,
                )
            u_sb = grp.tile([128, 16], F32, tag="u_sb", name=f"u_{g}")
            nc.scalar.activation(u_sb[:], z_sb[:], AF.Erf, scale=INV_SQRT2)
            etg_sb = grp.tile([128, 16], F32, tag="etg_sb", name=f"etg_{g}")
            nc.vector.scalar_tensor_tensor(
                out=etg_sb[:],
                in0=u_sb[:],
                scalar=1.0,
                in1=z_sb[:],
                op0=ALU.add,
                op1=ALU.mult,
            )
            for kb in range(2):
                nc.tensor.matmul(
                    psum_g[:, :],
                    etg_sb[:, kb * 8 : (kb + 1) * 8],
                    watt_sb[:, kb * 400 : (kb + 1) * 400],
                    start=False,
                    stop=(kb == 1),
                    skip_group_check=True,
                )
            logits_sb = grp.tile([GW, 400], F32, tag="logits", name=f"logits_{g}")
            nc.vector.tensor_tensor(
                out=logits_sb[:], in0=psum_g[:], in1=battr_sb[:], op=ALU.add
            )
            gate_g = grp.tile([GW, 400], F32, tag="gate_g", name=f"gate_{g}")
            nc.scalar.activation(gate_g[:], logits_sb[:], AF.Sigmoid)
            nc.sync.dma_start(o_gate[g0 : g0 + GW, :], gate_g[:])

            zap_g = grp.tile([GW, N], F32, tag="zap_g", name=f"zap_{g}")
            wsel_g = grp.tile([GW, N], F32, tag="wsel_g", name=f"wsel_{g}")
            woutv_g = grp.tile([GW, N], F32, tag="woutv_g", name=f"woutv_{g}")
            m8_g = grp.tile([GW, 16], F32, tag="m8_g", name=f"m8_{g}")
            nc.vector.max(out=m8_g[:, 0:8], in_=gate_g[:])
            nc.vector.match_replace(
                out=zap_g[:], in_to_replace=m8_g[:, 0:8], in_values=gate_g[:],
                imm_value=0.0,
            )
            nc.vector.max(out=m8_g[:, 8:16], in_=zap_g[:])
            nc.vector.memset(m8_g[:, 10:16], 0.0)
            nc.vector.match_replace(
                out=zap_g[:], in_to_replace=m8_g[:, 8:16], in_values=zap_g[:],
                imm_value=0.0,
            )
            nc.vector.tensor_tensor(
                out=wsel_g[:], in0=gate_g[:], in1=zap_g[:], op=ALU.subtract
            )
            nc.vector.tensor_scalar(
                out=woutv_g[:], in0=gate_g[:], scalar1=INV_N, scalar2=None,
                op0=ALU.mult,
            )

            psum_wt = ps_wt.tile([NP, NC, 2, GW], F32, tag="psum_wt", name=f"pswt_{g}")
            for c in range(NC):
                nc.tensor.matmul(
                    psum_wt[:, c, 0, :],
                    woutv_g[:, c * NP : (c + 1) * NP],
                    ident_sb[0:GW, 0:GW],
                    is_transpose=True,
                    start=(c == 0),
                    stop=True,
                    skip_group_check=True,
                )
                nc.tensor.matmul(
                    psum_wt[:, c, 1, :],
                    wsel_g[:, c * NP : (c + 1) * NP],
                    ident_sb[0:GW, 0:GW],
                    is_transpose=True,
                    start=False,
                    stop=True,
                    skip_group_check=True,
                )
            for half in range(2):
                wpad = wpad_tiles[(g % 2) * 2 + half]
                for ri in range(4):
                    if ri % 2 == 0:
                        nc.scalar.copy(
                            wpad[:, ri, :, 2 * ri : 2 * ri + 2],
                            psum_wt[:, :, :, half * 4 + ri],
                        )
                    else:
                        nc.vector.tensor_scalar(
                            out=wpad[:, ri, :, 2 * ri : 2 * ri + 2],
                            in0=psum_wt[:, :, :, half * 4 + ri],
                            scalar1=0.0,
                            scalar2=None,
                            op0=ALU.add,
                        )

            for half in range(2):
                psum_o = ps_o.tile([GW, H], F32, tag="psum_o", name=f"pso_{g}_{half}")
                wpad = wpad_tiles[(g % 2) * 2 + half]
                first = True
                for ri in range(4):
                    w = g0 + half * 4 + ri
                    pair, t = divmod(w, 2)
                    xr = xr_tiles[pair]
                    for c in range(NC):
                        nc.tensor.matmul(
                            psum_o[:, :],
                            wpad[:, ri, c, :],
                            xr[:, t * 1024 + c * 256 : t * 1024 + (c + 1) * 256],
                            start=first,
                            stop=(ri == 3 and c == NC - 1),
                            skip_group_check=True,
                        )
                        first = False
                out_sb = grp.tile([GW, H], F32, tag="out_sb",
                                  name=f"out_sb_{g}_{half}")
                nc.scalar.copy(out_sb[:], psum_o[:])
                w0 = g0 + half * 4
                nc.sync.dma_start(o_out01[2 * w0 : 2 * w0 + 8, :], out_sb[:])

        # software pipeline: group g-1's phase emitted mid-way through group g
        for g in range(NGROUPS):
            emit_pairs(g, phase_of=g - 1 if g > 0 else None)
        emit_phase(NGROUPS - 1)

    nc.finalize()
    return nc


def _host_prep(W_embed, b_embed, bn_gamma, bn_beta, bn_mean, bn_var, W_attend, b_attend):
    s = (bn_gamma / np.sqrt(bn_var + BN_EPS)).astype(np.float32)
    Wp = (W_embed * s[None, :]).astype(np.float32)          # [H, H]
    bp = (b_embed * s + bn_beta - bn_mean * s).astype(np.float32)  # [H]
    wt = np.ascontiguousarray(
        Wp.reshape(2, 128, 256).transpose(1, 0, 2).reshape(128, 512)
    )
    bpp = np.ascontiguousarray(bp.reshape(2, 128).T)        # [128, 2]
    watt = np.ascontiguousarray(
        (0.5 * W_attend).reshape(2, 128, 400).transpose(1, 0, 2).reshape(128, 800)
    )
    battr = np.ascontiguousarray(np.tile(b_attend[None, :], (GW, 1)).astype(np.float32))
    ident = np.eye(128, dtype=np.float32)
    sel4 = np.ascontiguousarray(
        np.tile(np.eye(4, dtype=np.float32).reshape(1, 16), (128, 1))
    )
    return wt, bpp, watt, battr, ident, sel4


def kernel(x, W_embed, b_embed, bn_gamma, bn_beta, bn_mean, bn_var, W_attend, b_attend):
    x = np.asarray(x, dtype=np.float32)
    wt, bpp, watt, battr, ident, sel4 = _host_prep(
        np.asarray(W_embed, np.float32), np.asarray(b_embed, np.float32),
        np.asarray(bn_gamma, np.float32), np.asarray(bn_beta, np.float32),
        np.asarray(bn_mean, np.float32), np.asarray(bn_var, np.float32),
        np.asarray(W_attend, np.float32), np.asarray(b_attend, np.float32),
    )

    if "nc" not in _CACHED:
        _CACHED["nc"] = _build_nc()
    nc = _CACHED["nc"]

    in_maps = []
    for b in range(B):
        in_maps.append({
            "xb": np.ascontiguousarray(x[b]),
            "wt": wt, "bpp": bpp, "watt": watt,
            "battr": battr, "ident": ident, "sel4": sel4,
            "zeros_in": np.zeros((128, 128), np.float32),
        })
    res = run_bass_kernel_spmd(nc, in_maps, list(range(B)))
    out01 = np.stack([r["o_out01"] for r in res.results], axis=0)  # [B, 64, H]
    out01 = out01.reshape(B, W, 2, H)
    output = np.ascontiguousarray(out01[:, :, 0, :])               # [B, W, H]
    selected = np.ascontiguousarray(out01[:, :, 1, :])             # [B, W, H]
    gate3 = np.stack([r["o_gate"] for r in res.results], axis=1)   # [W, B, N]
    return output, selected, gate3
